# revision 1
# baseline (speedup 1.0000x reference)
"""Trainium2 Bass kernel for nn_Cross_Attention_Block_3624952397825.

Mathematical structure exploited: the reference takes ``out[:, -1, :]`` --
the attention output of the LAST query token. That token comes from the
zero row appended by ``jnp.pad`` AFTER the conv stack, so its query vector
is exactly zero, its attention scores are exactly zero, and softmax over
exact zeros is exactly uniform (1/4096).  Hence

    bins[b] = mean_k V[b, k, :] = (mean_k lidar[b, k, :]) @ wv
    out[b]  = MLP3(leaky_relu chain)(bins[b])

The conv block, Q/K projections, and softmax are structurally dead code
for ANY input values.  Additionally there is no nonlinearity between wv
and wo1, so W1 = wv @ wo1 [256, 128] is constant-folded on the host.

Per core (2 batches): stream lidar as fp16 [128, 4096] tiles (8 KiB per
partition -> full single-queue DMA rate), reduce the 4096 points with
ones^T @ tile matmuls on TensorE (fp16 x fp16 products are exact for a
1.0 stationary; accumulation is fp32 in PSUM), then a tiny fp16 MLP
(fp32 biases, fp32 final add).  Weights ride the second HWDGE queue
(ScalarE) so the lidar FIFO is never interrupted; batch 0 streams first
so its MLP overlaps batch 1's DMA.  Measured model error ~6e-4.
"""

import numpy as np

B, NPTS, CH, DM = 16, 4096, 256, 1024
N_CORES = 8
BL = B // N_CORES            # batches per core
P = 128
TILE_F = 4096                # free dim of lidar tiles (16 pts x 256 ch)
N_TILES = NPTS * CH // (P * TILE_F)   # 2 tiles per batch

# fp16 weight pack layout (free dim)
OFF_W1 = 0                   # 2 k-chunks x 128   (W1 = wv @ wo1)
OFF_WO2 = 256                # 128
OFF_WO3 = 384                # 256  (stored [K=128, 256] for row-form output)
OFF_ONE16 = 640              # fp16 ones column
W16_F = 641
# fp32 pack columns
C_B1, C_B2 = 0, 1
W32_F = 4

_CACHE = {}


def _build_program():
    import concourse.bacc as bacc
    import concourse.mybir as mybir
    from concourse.tile import TileContext

    f32 = mybir.dt.float32
    f16 = mybir.dt.float16
    Alu = mybir.AluOpType
    Act = mybir.ActivationFunctionType

    nc = bacc.Bacc("TRN2")
    lidar = nc.dram_tensor("lidar16", [BL, NPTS, CH], f16, kind="ExternalInput")
    wp16d = nc.dram_tensor("wp16", [P, W16_F], f16, kind="ExternalInput")
    wp32d = nc.dram_tensor("wp32", [P, W32_F], f32, kind="ExternalInput")
    b3rowd = nc.dram_tensor("b3row", [1, CH], f32, kind="ExternalInput")
    out_rows = nc.dram_tensor("out_rows", [BL, CH], f32, kind="ExternalOutput")

    # [BL, 4096, 256] -> [(b t), 128, 4096]; 8 KiB contiguous per partition.
    lv = lidar[:, :, :].rearrange("b (t p q) c -> (b t) p (q c)", p=P, q=16)

    with TileContext(nc) as tc:
        with (
            tc.tile_pool(name="w", bufs=1) as wpool,
            tc.tile_pool(name="io", bufs=4) as iopool,
            tc.tile_pool(name="small", bufs=1) as spool,
            tc.tile_pool(name="ps", bufs=2, space="PSUM") as pspool,
            tc.tile_pool(name="orp", bufs=2, space="PSUM") as orpool,
            tc.tile_pool(name="mm", bufs=3, space="PSUM") as mmpool,
        ):
            # weights on the ScalarE HWDGE queue; lidar owns the SP queue
            wp16 = wpool.tile([P, W16_F], f16, tag="wp16")
            nc.scalar.dma_start(out=wp16[:, :], in_=wp16d[:, :])
            wp32 = wpool.tile([P, W32_F], f32, tag="wp32")
            nc.scalar.dma_start(out=wp32[:, :], in_=wp32d[:, :])
            b3row = wpool.tile([1, CH], f32, tag="b3row")
            nc.scalar.dma_start(out=b3row[:, :], in_=b3rowd[:, :])
            ones16 = wp16[:, OFF_ONE16:OFF_ONE16 + 1]

            for b in range(BL):
                # ---- point reduction: ones^T @ tile on TensorE ----
                # fp16 x 1.0 products are exact; fp32 PSUM accumulation.
                # 512-wide moving operand (2 points x 256 ch per matmul).
                MM_F = 2 * CH
                sred = pspool.tile([1, MM_F], f32, tag="sred")
                nmm = N_TILES * (TILE_F // MM_F)
                i = 0
                for t in range(N_TILES):
                    tin = iopool.tile([P, TILE_F], f16, tag="tin")
                    nc.sync.dma_start(out=tin[:, :], in_=lv[b * N_TILES + t, :, :])
                    for j in range(TILE_F // MM_F):
                        nc.tensor.matmul(sred[:, :], lhsT=ones16,
                                         rhs=tin[:, j * MM_F:(j + 1) * MM_F],
                                         start=(i == 0), stop=(i == nmm - 1))
                        i += 1
                # fold [1, 512] -> fp16 [1, 256] sums via SBUF bounce
                s512 = spool.tile([1, MM_F], f32, tag=f"s512{b}")
                nc.scalar.copy(out=s512[:, :], in_=sred[:, :])
                s16 = spool.tile([1, CH], f16, tag=f"s16{b}")
                nc.vector.tensor_add(out=s16[:, :], in0=s512[0:1, 0:CH],
                                     in1=s512[0:1, CH:MM_F])
                # transpose row [1, 256] -> 2 x [128, 1] via K=1 fp16 matmuls;
                # mean scale (1/4096, exact power of two) folded into the copy
                mt = []
                for k in range(2):
                    mtp = mmpool.tile([P, 1], f32, tag="mm")
                    nc.tensor.matmul(mtp[:, :], lhsT=s16[0:1, k * P:(k + 1) * P],
                                     rhs=ones16[0:1, 0:1], start=True, stop=True)
                    mt16 = spool.tile([P, 1], f16, tag=f"mt{b}{k}")
                    nc.scalar.activation(mt16[:, :], mtp[:, :], Act.Copy,
                                         scale=float(1.0 / NPTS))
                    mt.append(mt16)

                def leaky(zp, bias_col, tag):
                    z = spool.tile([P, 1], f16, tag=f"z{tag}")
                    nc.scalar.activation(z[:, :], zp[:, :], Act.Identity,
                                         bias=wp32[:, bias_col:bias_col + 1], scale=1.0)
                    h = spool.tile([P, 1], f16, tag=f"h{tag}")
                    nc.vector.scalar_tensor_tensor(out=h[:, :], in0=z[:, :], scalar=0.01,
                                                   in1=z[:, :], op0=Alu.mult, op1=Alu.max)
                    return h

                # h1 = leaky(m @ W1 + b1), W1 pre-folded on host
                h1p = mmpool.tile([P, 1], f32, tag="mm")
                for k in range(2):
                    nc.tensor.matmul(h1p[:, :],
                                     lhsT=wp16[:, OFF_W1 + k * P: OFF_W1 + (k + 1) * P],
                                     rhs=mt[k][:, :], start=(k == 0), stop=(k == 1))
                h1 = leaky(h1p, C_B1, f"1{b}")

                h2p = mmpool.tile([P, 1], f32, tag="mm")
                nc.tensor.matmul(h2p[:, :], lhsT=wp16[:, OFF_WO2:OFF_WO2 + P],
                                 rhs=h1[:, :], start=True, stop=True)
                h2 = leaky(h2p, C_B2, f"2{b}")

                # final layer in row form: h2^T @ wo3 -> [1, 256]
                orp = orpool.tile([1, CH], f32, tag="orp")
                nc.tensor.matmul(orp[:, :], lhsT=h2[:, :],
                                 rhs=wp16[:, OFF_WO3:OFF_WO3 + CH],
                                 start=True, stop=True)
                orow = spool.tile([1, CH], f32, tag=f"orow{b}")
                nc.vector.tensor_add(out=orow[:, :], in0=orp[:, :], in1=b3row[:, :])
                nc.scalar.dma_start(out=out_rows[b:b + 1, :], in_=orow[:, :])

    nc.compile()
    return nc


def _pack_weights(inputs):
    wv = np.asarray(inputs["wv"], np.float64)
    wo1 = np.asarray(inputs["wo1"], np.float64)
    W1 = (wv @ wo1)                           # [256, 128], no nonlinearity between

    wp16 = np.zeros((P, W16_F), np.float16)
    wp16[:, OFF_W1:OFF_W1 + P] = W1[0:128, :]
    wp16[:, OFF_W1 + P:OFF_W1 + 2 * P] = W1[128:256, :]
    wp16[:, OFF_WO2:OFF_WO2 + P] = np.asarray(inputs["wo2"], np.float32)
    wp16[:, OFF_WO3:OFF_WO3 + CH] = np.asarray(inputs["wo3"], np.float32)
    wp16[:, OFF_ONE16] = 1.0

    wp32 = np.zeros((P, W32_F), np.float32)
    wp32[:, C_B1] = np.asarray(inputs["b1"], np.float32)
    wp32[:, C_B2] = np.asarray(inputs["b2"], np.float32)
    b3row = np.asarray(inputs["b3"], np.float32).reshape(1, CH)
    return wp16, wp32, b3row


def kernel(**inputs):
    from concourse.bass_utils import run_bass_kernel_spmd

    if "nc" not in _CACHE:
        _CACHE["nc"] = _build_program()
    nc = _CACHE["nc"]

    lidar16 = np.ascontiguousarray(
        np.asarray(inputs["lidar"], dtype=np.float32).astype(np.float16))
    wp16, wp32, b3row = _pack_weights(inputs)

    in_maps = [
        {"lidar16": lidar16[i * BL:(i + 1) * BL], "wp16": wp16,
         "wp32": wp32, "b3row": b3row}
        for i in range(N_CORES)
    ]
    res = run_bass_kernel_spmd(nc, in_maps, list(range(N_CORES)),
                               **_CACHE.get("run_kwargs", {}))
    _CACHE["last_results"] = res
    out = np.concatenate([res.results[i]["out_rows"] for i in range(N_CORES)], axis=0)
    return np.ascontiguousarray(out, dtype=np.float32)



# revision 9
# speedup vs baseline: 1.0823x; 1.0823x over previous
"""Trainium2 Bass kernel for nn_Cross_Attention_Block_3624952397825.

Mathematical structure exploited: the reference takes ``out[:, -1, :]`` --
the attention output of the LAST query token. That token comes from the
zero row appended by ``jnp.pad`` AFTER the conv stack, so its query vector
is exactly zero, its attention scores are exactly zero, and softmax over
exact zeros is exactly uniform (1/4096).  Hence

    bins[b] = mean_k V[b, k, :] = (mean_k lidar[b, k, :]) @ wv
    out[b]  = MLP3(leaky_relu chain)(bins[b])

The conv block, Q/K projections, and softmax are structurally dead code
for ANY input values.  There is no nonlinearity between wv and wo1, so
W1 = (wv @ wo1) / NPTS is constant-folded on the host (the 1/4096 mean
scale riding along).

Implementation (per core, 2 batches, fp8 e3m4 lidar = 2 MiB streamed):
  * Point-major tiles [128 pts, (q c)] reduced on TensorE via ones^T
    matmuls (fp8e3 runs at 1 cycle/row, fp32 PSUM accumulation is exact).
    A few warm-up matmuls ramp the PE out of its low p-state before real
    data lands.
  * Channel-major tiles [128 ch, pts]: DVE tensor_reduce handles channel
    half 0, ScalarE activation(accum_out=) handles half 1 -- both produce
    [128, 1] column partial sums directly (no transpose needed).
  * Tail is 2-wide across batches: fold + K=1 transpose matmuls + 3-layer
    MLP with hardware Lrelu, one [2, 256] output DMA.
fp8 quantization of lidar gives rel err ~1.2e-2 (< 2e-2 gate); everything
downstream of the sums is fp16/fp32.
"""

import numpy as np
import ml_dtypes

B, NPTS, CH = 16, 4096, 256
N_CORES = 8
BL = B // N_CORES            # batches per core
P = 128

# Split of the 4096 points per batch:
#   X_PE points  -> point-major tiles for TensorE
#   X_V  points  -> channel-major tiles for DVE (ch 0-127) + ScalarE (ch 128-255)
Q_PE = 22                    # points per partition in the PE tiles
X_PE = P * Q_PE              # 2816
X_V = NPTS - X_PE            # 1280
PE_F = Q_PE * CH             # 5632 free elements per PE tile
QA = 12                      # first PE chunk, in q units (free = QA*256)
FA = QA * CH                 # 3072
FB = PE_F - FA               # 2560
MM_F = 2 * CH                # 512-wide moving operand per matmul
N_WARM = 5                   # PE p-state warm-up matmuls

# fp16 weight pack layout (free dim of wpk)
OFF_W1 = 0                   # 2 k-chunks x 128   (W1 = wv @ wo1 / NPTS)
OFF_WO2 = 256                # 128
OFF_WO3 = 384                # 256
OFF_ONE = 640                # fp16 1.0 column
OFF_B = 642                  # 4 fp16 cols = [b1, b2] bitcast fp32 [128, 2]
WPK_F = 648

_CACHE = {}


def _build_program():
    import concourse.bacc as bacc
    import concourse.mybir as mybir
    from concourse.tile import TileContext

    f32 = mybir.dt.float32
    f16 = mybir.dt.float16
    f8 = mybir.dt.float8e3
    Alu = mybir.AluOpType
    Act = mybir.ActivationFunctionType
    Ax = mybir.AxisListType

    nc = bacc.Bacc("TRN2")
    lpe = nc.dram_tensor("lidar_pe", [BL, P, PE_F], f8, kind="ExternalInput")
    lch = nc.dram_tensor("lidar_ch", [BL, P, 2 * X_V], f8, kind="ExternalInput")
    wpkd = nc.dram_tensor("wpk", [P, WPK_F], f16, kind="ExternalInput")
    b3d = nc.dram_tensor("b3row2", [BL, CH], f32, kind="ExternalInput")
    out_rows = nc.dram_tensor("out_rows", [BL, CH], f32, kind="ExternalOutput")

    with TileContext(nc) as tc:
        with (
            tc.tile_pool(name="w", bufs=1) as wpool,
            tc.tile_pool(name="io", bufs=1) as iopool,
            tc.tile_pool(name="small", bufs=1) as spool,
            tc.tile_pool(name="acc", bufs=1, space="PSUM") as accpool,
            tc.tile_pool(name="mt", bufs=1, space="PSUM") as mtpool,
            tc.tile_pool(name="mm", bufs=1, space="PSUM") as mmpool,
        ):
            # ---- weights on the Scalar HWDGE queue; lidar owns the SP queue
            wpk = wpool.tile([P, WPK_F], f16, tag="wpk")
            nc.scalar.dma_start(out=wpk[:, :], in_=wpkd[:, :])
            b3sb = wpool.tile([BL, CH], f32, tag="b3")
            nc.scalar.dma_start(out=b3sb[:, :], in_=b3d[:, :])
            ones = wpk[:, OFF_ONE:OFF_ONE + 1]
            biases = wpk[:, OFF_B:OFF_B + 4].bitcast(f32)  # [128, 2] = b1,b2
            ones8 = wpool.tile([P, 1], f8, tag="ones8")
            nc.vector.memset(ones8[:, :], 1.0)

            # ---- PE p-state warm-up: garbage matmuls on a zeroed tile
            warm = wpool.tile([P, MM_F], f16, tag="warm")
            nc.vector.memset(warm[:, :], 0.0)
            wps = mmpool.tile([1, MM_F], f32, tag="warm")
            for _ in range(N_WARM):
                nc.tensor.matmul(wps[:, :], lhsT=warm[:, 0:1], rhs=warm[:, :],
                                 start=True, stop=True)

            # ---- the full lidar stream, issued up-front on the sync queue
            pe_t, ch_t = [], []
            for b in range(BL):
                ta = iopool.tile([P, FA], f8, tag=f"pea{b}")
                nc.sync.dma_start(out=ta[:, :], in_=lpe[b, :, 0:FA])
                tc_ = iopool.tile([P, 2 * X_V], f8, tag=f"ch{b}")
                nc.sync.dma_start(out=tc_[:, :], in_=lch[b, :, :])
                tb = iopool.tile([P, FB], f8, tag=f"peb{b}")
                nc.sync.dma_start(out=tb[:, :], in_=lpe[b, :, FA:PE_F])
                pe_t.append((ta, tb))
                ch_t.append(tc_)

            # ---- per-batch reductions
            m16 = [spool.tile([P, BL], f16, tag=f"m16{k}", name=f"m16{k}")
                   for k in range(2)]
            for b in range(BL):
                ta, tb = pe_t[b]
                acc = accpool.tile([1, MM_F], f32, tag=f"acc{b}")
                nmm = PE_F // MM_F
                i = 0
                for t, nf in ((ta, FA), (tb, FB)):
                    for j in range(nf // MM_F):
                        nc.tensor.matmul(acc[:, :], lhsT=ones8[:, :],
                                         rhs=t[:, j * MM_F:(j + 1) * MM_F],
                                         start=(i == 0), stop=(i == nmm - 1))
                        i += 1
                # channel-major partial sums: DVE takes ch 0-127, ACT 128-255
                dv = spool.tile([P, 1], f32, tag=f"dv{b}")
                nc.vector.tensor_reduce(out=dv[:, :], in_=ch_t[b][:, 0:X_V],
                                        axis=Ax.X, op=Alu.add)
                sc = spool.tile([P, 1], f32, tag=f"sc{b}")
                nc.scalar.activation(ch_t[b][:, X_V:2 * X_V],
                                     ch_t[b][:, X_V:2 * X_V], Act.Copy,
                                     accum_out=sc[:, :])
                # fold the (q even | q odd) interleave of the PE partial
                # (PSUM -> SBUF bounce first: only one PSUM input per DVE op)
                s512 = spool.tile([1, MM_F], f16, tag=f"s512{b}", name=f"s512{b}")
                nc.scalar.copy(out=s512[:, :], in_=acc[:, :])
                s16 = spool.tile([1, CH], f16, tag=f"s16{b}")
                nc.vector.tensor_add(out=s16[:, :], in0=s512[0:1, 0:CH],
                                     in1=s512[0:1, CH:MM_F])
                # transpose row -> columns via K=1 matmuls, add ch-major parts
                for k in range(2):
                    mtp = mtpool.tile([P, 1], f32, tag=f"mt{k}", name=f"mt{k}")
                    nc.tensor.matmul(mtp[:, :], lhsT=s16[0:1, k * P:(k + 1) * P],
                                     rhs=ones[0:1, 0:1], start=True, stop=True)
                    part = dv if k == 0 else sc
                    nc.vector.tensor_add(out=m16[k][:, b:b + 1],
                                         in0=mtp[:, :], in1=part[:, :])

            # ---- joint MLP for both batches, [128, BL] wide
            h1p = mmpool.tile([P, BL], f32, tag="mlp", name="h1p")
            for k in range(2):
                nc.tensor.matmul(h1p[:, :],
                                 lhsT=wpk[:, OFF_W1 + k * P:OFF_W1 + (k + 1) * P],
                                 rhs=m16[k][:, :], start=(k == 0), stop=(k == 1))
            z1 = spool.tile([P, BL], f16, tag="z1")
            nc.scalar.activation(z1[:, :], h1p[:, :], Act.Lrelu,
                                 bias=biases[:, 0:1], scale=1.0, alpha=0.01)
            h2p = mmpool.tile([P, BL], f32, tag="mlp", name="h2p")
            nc.tensor.matmul(h2p[:, :], lhsT=wpk[:, OFF_WO2:OFF_WO2 + P],
                             rhs=z1[:, :], start=True, stop=True)
            z2 = spool.tile([P, BL], f16, tag="z2")
            nc.scalar.activation(z2[:, :], h2p[:, :], Act.Lrelu,
                                 bias=biases[:, 1:2], scale=1.0, alpha=0.01)
            orp = mmpool.tile([BL, CH], f32, tag="orp")
            nc.tensor.matmul(orp[:, :], lhsT=z2[:, :],
                             rhs=wpk[:, OFF_WO3:OFF_WO3 + CH],
                             start=True, stop=True)
            orow = spool.tile([BL, CH], f32, tag="orow")
            nc.vector.tensor_add(out=orow[:, :], in0=orp[:, :], in1=b3sb[:, :])
            nc.scalar.dma_start(out=out_rows[:, :], in_=orow[:, :])

    nc.compile()
    return nc


def _pack_inputs(inputs):
    f8 = ml_dtypes.float8_e3m4
    lidar8 = np.asarray(inputs["lidar"], np.float32).astype(f8)  # [16,4096,256]
    # point-major PE tiles: partition p holds points p*Q_PE .. p*Q_PE+Q_PE-1
    pe8 = np.ascontiguousarray(
        lidar8[:, :X_PE].reshape(B, P, PE_F))
    # channel-major tiles: [b, p, (h q)] = lidar[b, point q of tail, ch h*128+p]
    ch8 = np.ascontiguousarray(
        lidar8[:, X_PE:].transpose(0, 2, 1).reshape(B, 2, P, X_V)
        .transpose(0, 2, 1, 3).reshape(B, P, 2 * X_V))

    wv = np.asarray(inputs["wv"], np.float64)
    wo1 = np.asarray(inputs["wo1"], np.float64)
    W1 = (wv @ wo1) / NPTS                    # [256, 128]; mean folded in

    wpk = np.zeros((P, WPK_F), np.float16)
    wpk[:, OFF_W1:OFF_W1 + P] = W1[0:P, :]
    wpk[:, OFF_W1 + P:OFF_W1 + 2 * P] = W1[P:2 * P, :]
    wpk[:, OFF_WO2:OFF_WO2 + P] = np.asarray(inputs["wo2"], np.float32)
    wpk[:, OFF_WO3:OFF_WO3 + CH] = np.asarray(inputs["wo3"], np.float32)
    wpk[:, OFF_ONE] = 1.0
    bb = np.stack([np.asarray(inputs["b1"], np.float32),
                   np.asarray(inputs["b2"], np.float32)], axis=1)  # [128, 2]
    wpk[:, OFF_B:OFF_B + 4] = bb.view(np.float16)
    b3row2 = np.broadcast_to(
        np.asarray(inputs["b3"], np.float32).reshape(1, CH), (BL, CH)).copy()
    return pe8, ch8, wpk, b3row2


def kernel(**inputs):
    from concourse.bass_utils import run_bass_kernel_spmd

    if "nc" not in _CACHE:
        _CACHE["nc"] = _build_program()
    nc = _CACHE["nc"]

    pe8, ch8, wpk, b3row2 = _pack_inputs(inputs)

    in_maps = [
        {"lidar_pe": pe8[i * BL:(i + 1) * BL],
         "lidar_ch": ch8[i * BL:(i + 1) * BL],
         "wpk": wpk, "b3row2": b3row2}
        for i in range(N_CORES)
    ]
    res = run_bass_kernel_spmd(nc, in_maps, list(range(N_CORES)),
                               **_CACHE.get("run_kwargs", {}))
    _CACHE["last_results"] = res
    out = np.concatenate([res.results[i]["out_rows"] for i in range(N_CORES)], axis=0)
    return np.ascontiguousarray(out, dtype=np.float32)


# revision 10
# speedup vs baseline: 1.2520x; 1.1567x over previous
"""Trainium2 Bass kernel for nn_Cross_Attention_Block_3624952397825.

Mathematical structure exploited: the reference takes ``out[:, -1, :]`` --
the attention output of the LAST query token. That token comes from the
zero row appended by ``jnp.pad`` AFTER the conv stack, so its query vector
is exactly zero, its attention scores are exactly zero, and softmax over
exact zeros is exactly uniform (1/4096).  Hence

    bins[b] = mean_k V[b, k, :] = (mean_k lidar[b, k, :]) @ wv
    out[b]  = MLP3(leaky_relu chain)(bins[b])

The conv block, Q/K projections, and softmax are structurally dead code
for ANY input values.  There is no nonlinearity between wv and wo1, so
W1 = (wv @ wo1) / NPTS is constant-folded on the host (the 1/4096 mean
scale riding along).

Implementation (per core, 2 batches, fp8 e3m4 lidar = 2 MiB streamed on
the single SP HWDGE queue):
  * Point-major tiles [128 pts, (q c)] reduced on TensorE via ones^T
    matmuls (fp8e3, fp32 PSUM accumulation).  Warm-up matmuls hold the
    PE busy through the preamble so it is out of its low p-state when
    real data lands.
  * Channel-major tiles [128 ch, pts]: DVE tensor_reduce handles channel
    half 0, ScalarE activation(accum_out=) handles half 1 -- both produce
    [128, 1] column partial sums directly (no transpose needed).
  * Tail is 2-wide across batches: fold + K=1 transpose matmuls + 3-layer
    MLP (ScalarE Identity+bias, DVE leaky), one [2, 256] output DMA.
  * Unused DMA queue sets (Pool SWDGE, Activation HWDGE) are dropped from
    the module so the NEFF prologue/teardown has fewer rings to init and
    drain -- the per-ring drain loop at program end is a large fixed cost.
fp8 quantization of lidar gives rel err ~1.2e-2 (< 2e-2 gate); everything
downstream of the sums is fp16/fp32.
"""

import numpy as np
import ml_dtypes

B, NPTS, CH = 16, 4096, 256
N_CORES = 8
BL = B // N_CORES            # batches per core
P = 128

# Per-batch split of the 4096 points:
#   X_PE point-major -> TensorE;  X_V channel-major -> DVE (ch 0-127) +
#   ScalarE (ch 128-255).  Rates (sustained): PE ~0.89 ns/row,
#   DVE ~1.39 ns/elem (fp8 1x), ACT ~1.27 ns/elem.
Q_PE = 14                    # points per partition in the PE region
X_PE = P * Q_PE              # 1792
X_V = NPTS - X_PE            # 2304
PE_F = Q_PE * CH             # 3584 free elements of PE data per batch
FA = 1024                    # first PE chunk (2 matmuls) for an early start
FB = PE_F - FA               # 2560 (5 matmuls)
CH_F = 2 * X_V               # 4608 free elements of channel-major data
TILE_F = PE_F + CH_F         # 8192 per batch (1 MiB fp8)
MM_F = 2 * CH                # 512-wide moving operand per matmul
N_WARM = 3                   # PE p-state warm-up matmuls
STRIP_QUEUES = True          # drop Pool/Act DMA queue sets from the module

# fp16 weight pack layout (free dim of wpk)
OFF_W1 = 0                   # 2 k-chunks x 128   (W1 = wv @ wo1 / NPTS)
OFF_WO2 = 256                # 128
OFF_WO3 = 384                # 256
OFF_B = 640                  # 4 fp16 cols = [b1, b2] bitcast fp32 [128, 2]
OFF_B3 = 644                 # rows 0-1, 512 fp16 cols = b3 bitcast [2, 256]
WPK_F = 1156

_CACHE = {}


def _build_program():
    import concourse.bacc as bacc
    import concourse.mybir as mybir
    from concourse.tile import TileContext

    f32 = mybir.dt.float32
    f16 = mybir.dt.float16
    f8 = mybir.dt.float8e3
    Alu = mybir.AluOpType
    Act = mybir.ActivationFunctionType
    Ax = mybir.AxisListType

    nc = bacc.Bacc("TRN2")
    if STRIP_QUEUES:
        nc.m.queues = [q for q in nc.m.queues if q.name == "qSPDynamicHW"]

    lid = nc.dram_tensor("lid8", [BL, P, TILE_F], f8, kind="ExternalInput")
    wpkd = nc.dram_tensor("wpk", [P, WPK_F], f16, kind="ExternalInput")
    out_rows = nc.dram_tensor("out_rows", [BL, CH], f32, kind="ExternalOutput")

    with TileContext(nc) as tc:
        with (
            tc.tile_pool(name="w", bufs=1) as wpool,
            tc.tile_pool(name="io", bufs=1) as iopool,
            tc.tile_pool(name="small", bufs=1) as spool,
            tc.tile_pool(name="acc", bufs=1, space="PSUM") as accpool,
            tc.tile_pool(name="mt", bufs=1, space="PSUM") as mtpool,
            tc.tile_pool(name="mm", bufs=1, space="PSUM") as mmpool,
        ):
            ones8 = wpool.tile([P, 1], f8, tag="ones8")
            nc.vector.memset(ones8[:, :], 1.0)
            ones16 = wpool.tile([1, 1], f16, tag="ones16")
            nc.vector.memset(ones16[:, :], 1.0)

            # ---- PE p-state warm-up: garbage matmuls on a zeroed tile
            warm = wpool.tile([P, MM_F], f16, tag="warm")
            nc.vector.memset(warm[:, :], 0.0)
            wps = mmpool.tile([1, MM_F], f32, tag="warm")
            for _ in range(N_WARM):
                nc.tensor.matmul(wps[:, :], lhsT=warm[:, 0:1], rhs=warm[:, :],
                                 start=True, stop=True)

            # ---- the full lidar stream on the SP queue; weights ride last
            tiles = []
            for b in range(BL):
                tpa = iopool.tile([P, FA], f8, tag=f"tpa{b}", name=f"tpa{b}")
                nc.sync.dma_start(out=tpa[:, :], in_=lid[b, :, 0:FA])
                tch = iopool.tile([P, CH_F], f8, tag=f"tch{b}", name=f"tch{b}")
                nc.sync.dma_start(out=tch[:, :], in_=lid[b, :, PE_F:TILE_F])
                tpb = iopool.tile([P, FB], f8, tag=f"tpb{b}", name=f"tpb{b}")
                nc.sync.dma_start(out=tpb[:, :], in_=lid[b, :, FA:PE_F])
                tiles.append((tpa, tpb, tch))
            wpk = wpool.tile([P, WPK_F], f16, tag="wpk")
            nc.sync.dma_start(out=wpk[:, :], in_=wpkd[:, :])
            biases = wpk[:, OFF_B:OFF_B + 4].bitcast(f32)    # [128, 2] b1,b2
            b3sb = wpk[0:2, OFF_B3:OFF_B3 + 2 * CH].bitcast(f32)  # [2, 256]

            # ---- per-batch reductions
            m16 = [spool.tile([P, BL], f16, tag=f"m16{k}", name=f"m16{k}")
                   for k in range(2)]
            for b in range(BL):
                tpa, tpb, tch = tiles[b]
                acc = accpool.tile([1, MM_F], f32, tag=f"acc{b}", name=f"acc{b}")
                nmm = PE_F // MM_F
                i = 0
                for t, nf in ((tpa, FA), (tpb, FB)):
                    for j in range(nf // MM_F):
                        nc.tensor.matmul(acc[:, :], lhsT=ones8[:, :],
                                         rhs=t[:, j * MM_F:(j + 1) * MM_F],
                                         start=(i == 0), stop=(i == nmm - 1))
                        i += 1
                # channel-major partial sums: DVE ch 0-127, ACT ch 128-255
                dv = spool.tile([P, 1], f32, tag=f"dv{b}", name=f"dv{b}")
                nc.vector.tensor_reduce(out=dv[:, :], in_=tch[:, 0:X_V],
                                        axis=Ax.X, op=Alu.add)
                sc = spool.tile([P, 1], f32, tag=f"sc{b}", name=f"sc{b}")
                nc.scalar.activation(tch[:, X_V:CH_F], tch[:, X_V:CH_F],
                                     Act.Copy, accum_out=sc[:, :])
                # fold the (q even | q odd) interleave of the PE partial
                s512 = spool.tile([1, MM_F], f16, tag=f"s512{b}", name=f"s512{b}")
                nc.scalar.copy(out=s512[:, :], in_=acc[:, :])
                s16 = spool.tile([1, CH], f16, tag=f"s16{b}", name=f"s16{b}")
                nc.vector.tensor_add(out=s16[:, :], in0=s512[0:1, 0:CH],
                                     in1=s512[0:1, CH:MM_F])
                # transpose row -> columns via K=1 matmuls, add ch-major parts
                for k in range(2):
                    mtp = mtpool.tile([P, 1], f32, tag=f"mt{k}", name=f"mt{k}")
                    nc.tensor.matmul(mtp[:, :], lhsT=s16[0:1, k * P:(k + 1) * P],
                                     rhs=ones16[0:1, 0:1], start=True, stop=True)
                    part = dv if k == 0 else sc
                    nc.vector.tensor_add(out=m16[k][:, b:b + 1],
                                         in0=mtp[:, :], in1=part[:, :])

            # ---- joint MLP for both batches, [128, BL] wide
            def leaky(hp, bias_col, tag):
                z = spool.tile([P, BL], f16, tag=f"z{tag}", name=f"z{tag}")
                nc.scalar.activation(z[:, :], hp[:, :], Act.Identity,
                                     bias=biases[:, bias_col:bias_col + 1],
                                     scale=1.0)
                h = spool.tile([P, BL], f16, tag=f"h{tag}", name=f"h{tag}")
                nc.vector.scalar_tensor_tensor(out=h[:, :], in0=z[:, :],
                                               scalar=0.01, in1=z[:, :],
                                               op0=Alu.mult, op1=Alu.max)
                return h

            h1p = mmpool.tile([P, BL], f32, tag="mlp", name="h1p")
            for k in range(2):
                nc.tensor.matmul(h1p[:, :],
                                 lhsT=wpk[:, OFF_W1 + k * P:OFF_W1 + (k + 1) * P],
                                 rhs=m16[k][:, :], start=(k == 0), stop=(k == 1))
            z1 = leaky(h1p, 0, "1")
            h2p = mmpool.tile([P, BL], f32, tag="mlp", name="h2p")
            nc.tensor.matmul(h2p[:, :], lhsT=wpk[:, OFF_WO2:OFF_WO2 + P],
                             rhs=z1[:, :], start=True, stop=True)
            z2 = leaky(h2p, 1, "2")
            orp = mmpool.tile([BL, CH], f32, tag="orp")
            nc.tensor.matmul(orp[:, :], lhsT=z2[:, :],
                             rhs=wpk[:, OFF_WO3:OFF_WO3 + CH],
                             start=True, stop=True)
            orow = spool.tile([BL, CH], f32, tag="orow")
            nc.vector.tensor_add(out=orow[:, :], in0=orp[:, :], in1=b3sb[:, :])
            nc.sync.dma_start(out=out_rows[:, :], in_=orow[:, :])

    nc.compile()
    return nc


def _pack_inputs(inputs):
    f8 = ml_dtypes.float8_e3m4
    lidar8 = np.asarray(inputs["lidar"], np.float32).astype(f8)  # [16,4096,256]
    # point-major region: partition p holds points p*Q_PE .. p*Q_PE+Q_PE-1
    pe8 = lidar8[:, :X_PE].reshape(B, P, PE_F)
    # channel-major region: [b, p, (h q)] = lidar[b, tail point q, ch h*128+p]
    ch8 = (lidar8[:, X_PE:].transpose(0, 2, 1).reshape(B, 2, P, X_V)
           .transpose(0, 2, 1, 3).reshape(B, P, CH_F))
    lid8 = np.ascontiguousarray(np.concatenate([pe8, ch8], axis=2))

    wv = np.asarray(inputs["wv"], np.float64)
    wo1 = np.asarray(inputs["wo1"], np.float64)
    W1 = (wv @ wo1) / NPTS                    # [256, 128]; mean folded in

    wpk = np.zeros((P, WPK_F), np.float16)
    wpk[:, OFF_W1:OFF_W1 + P] = W1[0:P, :]
    wpk[:, OFF_W1 + P:OFF_W1 + 2 * P] = W1[P:2 * P, :]
    wpk[:, OFF_WO2:OFF_WO2 + P] = np.asarray(inputs["wo2"], np.float32)
    wpk[:, OFF_WO3:OFF_WO3 + CH] = np.asarray(inputs["wo3"], np.float32)
    bb = np.stack([np.asarray(inputs["b1"], np.float32),
                   np.asarray(inputs["b2"], np.float32)], axis=1)  # [128, 2]
    wpk[:, OFF_B:OFF_B + 4] = bb.view(np.float16)
    b3h = np.asarray(inputs["b3"], np.float32).reshape(1, CH).view(np.float16)
    wpk[0:2, OFF_B3:OFF_B3 + 2 * CH] = np.broadcast_to(b3h, (2, 2 * CH))
    return lid8, wpk


def kernel(**inputs):
    from concourse.bass_utils import run_bass_kernel_spmd

    if "nc" not in _CACHE:
        _CACHE["nc"] = _build_program()
    nc = _CACHE["nc"]

    lid8, wpk = _pack_inputs(inputs)

    in_maps = [
        {"lid8": lid8[i * BL:(i + 1) * BL], "wpk": wpk}
        for i in range(N_CORES)
    ]
    res = run_bass_kernel_spmd(nc, in_maps, list(range(N_CORES)),
                               **_CACHE.get("run_kwargs", {}))
    _CACHE["last_results"] = res
    out = np.concatenate([res.results[i]["out_rows"] for i in range(N_CORES)], axis=0)
    return np.ascontiguousarray(out, dtype=np.float32)


# revision 13
# speedup vs baseline: 1.2522x; 1.0002x over previous
"""Trainium2 Bass kernel for nn_Cross_Attention_Block_3624952397825.

Mathematical structure exploited: the reference takes ``out[:, -1, :]`` --
the attention output of the LAST query token. That token comes from the
zero row appended by ``jnp.pad`` AFTER the conv stack, so its query vector
is exactly zero, its attention scores are exactly zero, and softmax over
exact zeros is exactly uniform (1/4096).  Hence

    bins[b] = mean_k V[b, k, :] = (mean_k lidar[b, k, :]) @ wv
    out[b]  = MLP3(leaky_relu chain)(bins[b])

The conv block, Q/K projections, and softmax are structurally dead code
for ANY input values.  There is no nonlinearity between wv and wo1, so
W1 = (wv @ wo1) / NPTS is constant-folded on the host (the 1/4096 mean
scale riding along).

Implementation (per core, 2 batches, fp8 e3m4 lidar = 2 MiB streamed on
the single SP HWDGE queue):
  * Point-major tiles [128 pts, (q c)] reduced on TensorE via ones^T
    matmuls (fp8e3, fp32 PSUM accumulation).  Warm-up matmuls hold the
    PE busy through the preamble so it is out of its low p-state when
    real data lands.
  * Channel-major tiles [128 ch, pts]: DVE tensor_reduce handles channel
    half 0, ScalarE activation(accum_out=) handles half 1 -- both produce
    [128, 1] column partial sums directly (no transpose needed).
  * Tail is 2-wide across batches: fold + K=1 transpose matmuls + 3-layer
    MLP (ScalarE Identity+bias, DVE leaky), one [2, 256] output DMA.
  * Unused DMA queue sets (Pool SWDGE, Activation HWDGE) are dropped from
    the module so the NEFF prologue/teardown has fewer rings to init and
    drain -- the per-ring drain loop at program end is a large fixed cost.
fp8 quantization of lidar gives rel err ~1.2e-2 (< 2e-2 gate); everything
downstream of the sums is fp16/fp32.
"""

import numpy as np
import ml_dtypes

B, NPTS, CH = 16, 4096, 256
N_CORES = 8
BL = B // N_CORES            # batches per core
P = 128

# Per-batch split of the 4096 points:
#   X_PE point-major -> TensorE;  X_V channel-major -> DVE (ch 0-127) +
#   ScalarE (ch 128-255).  Rates (sustained): PE ~0.89 ns/row,
#   DVE ~1.39 ns/elem (fp8 1x), ACT ~1.27 ns/elem.
Q_PE = 14                    # points per partition in the PE region
X_PE = P * Q_PE              # 1792
X_V = NPTS - X_PE            # 2304
PE_F = Q_PE * CH             # 3584 free elements of PE data per batch
CH_F = 2 * X_V               # 4608 free elements of channel-major data
TILE_F = PE_F + CH_F         # 8192 per batch (1 MiB fp8)
MM_F = 2 * CH                # 512-wide moving operand per matmul
N_WARM = 6                   # PE p-state warm-up matmuls
STRIP_QUEUES = True          # drop Pool/Act DMA queue sets from the module

# fp16 weight pack layout (free dim of wpk)
OFF_W1 = 0                   # 2 k-chunks x 128   (W1 = wv @ wo1 / NPTS)
OFF_WO2 = 256                # 128
OFF_WO3 = 384                # 256
OFF_B = 640                  # 4 fp16 cols = [b1, b2] bitcast fp32 [128, 2]
OFF_B3 = 644                 # rows 0-1, 512 fp16 cols = b3 bitcast [2, 256]
WPK_F = 1156

_CACHE = {}


def _build_program():
    import concourse.bacc as bacc
    import concourse.mybir as mybir
    from concourse.tile import TileContext

    f32 = mybir.dt.float32
    f16 = mybir.dt.float16
    f8 = mybir.dt.float8e3
    Alu = mybir.AluOpType
    Act = mybir.ActivationFunctionType
    Ax = mybir.AxisListType

    nc = bacc.Bacc("TRN2")
    if STRIP_QUEUES:
        nc.m.queues = [q for q in nc.m.queues if q.name == "qSPDynamicHW"]

    lid = nc.dram_tensor("lid8", [BL, P, TILE_F], f8, kind="ExternalInput")
    wpkd = nc.dram_tensor("wpk", [P, WPK_F], f16, kind="ExternalInput")
    out_rows = nc.dram_tensor("out_rows", [BL, CH], f32, kind="ExternalOutput")

    with TileContext(nc) as tc:
        with (
            tc.tile_pool(name="w", bufs=1) as wpool,
            tc.tile_pool(name="io", bufs=1) as iopool,
            tc.tile_pool(name="small", bufs=1) as spool,
            tc.tile_pool(name="acc", bufs=1, space="PSUM") as accpool,
            tc.tile_pool(name="mt", bufs=1, space="PSUM") as mtpool,
            tc.tile_pool(name="mm", bufs=1, space="PSUM") as mmpool,
        ):
            # memsets on GpSimd: it is free right after the preamble, while
            # DVE/ACT are still loading their engine tables
            ones8 = wpool.tile([P, 1], f8, tag="ones8")
            nc.gpsimd.memset(ones8[:, :], 1.0)
            ones16 = wpool.tile([1, 1], f16, tag="ones16")
            nc.gpsimd.memset(ones16[:, :], 1.0)

            # ---- PE p-state warm-up: garbage matmuls on a zeroed tile
            warm = wpool.tile([P, MM_F], f16, tag="warm")
            nc.gpsimd.memset(warm[:, :], 0.0)
            wps = mmpool.tile([1, MM_F], f32, tag="warm")
            for _ in range(N_WARM):
                nc.tensor.matmul(wps[:, :], lhsT=warm[:, 0:1], rhs=warm[:, :],
                                 start=True, stop=True)

            # ---- the full lidar stream on the SP queue; weights ride last
            tiles = []
            for b in range(BL):
                tpe = iopool.tile([P, PE_F], f8, tag=f"tpe{b}", name=f"tpe{b}")
                nc.sync.dma_start(out=tpe[:, :], in_=lid[b, :, 0:PE_F])
                tch = iopool.tile([P, CH_F], f8, tag=f"tch{b}", name=f"tch{b}")
                nc.sync.dma_start(out=tch[:, :], in_=lid[b, :, PE_F:TILE_F])
                tiles.append((tpe, tch))
            wpk = wpool.tile([P, WPK_F], f16, tag="wpk")
            nc.sync.dma_start(out=wpk[:, :], in_=wpkd[:, :])
            biases = wpk[:, OFF_B:OFF_B + 4].bitcast(f32)    # [128, 2] b1,b2
            b3sb = wpk[0:2, OFF_B3:OFF_B3 + 2 * CH].bitcast(f32)  # [2, 256]

            # ---- per-batch reductions
            m16 = [spool.tile([P, BL], f16, tag=f"m16{k}", name=f"m16{k}")
                   for k in range(2)]
            for b in range(BL):
                tpe, tch = tiles[b]
                acc = accpool.tile([1, MM_F], f32, tag=f"acc{b}", name=f"acc{b}")
                nmm = PE_F // MM_F
                for j in range(nmm):
                    nc.tensor.matmul(acc[:, :], lhsT=ones8[:, :],
                                     rhs=tpe[:, j * MM_F:(j + 1) * MM_F],
                                     start=(j == 0), stop=(j == nmm - 1))
                # channel-major partial sums: DVE ch 0-127, ACT ch 128-255
                dv = spool.tile([P, 1], f32, tag=f"dv{b}", name=f"dv{b}")
                nc.vector.tensor_reduce(out=dv[:, :], in_=tch[:, 0:X_V],
                                        axis=Ax.X, op=Alu.add)
                sc = spool.tile([P, 1], f32, tag=f"sc{b}", name=f"sc{b}")
                nc.scalar.activation(tch[:, X_V:CH_F], tch[:, X_V:CH_F],
                                     Act.Copy, accum_out=sc[:, :])
                # PSUM -> SBUF bounce, split across ACT and DVE in parallel
                sA = spool.tile([1, CH], f16, tag=f"sA{b}", name=f"sA{b}")
                nc.scalar.copy(out=sA[:, :], in_=acc[0:1, 0:CH])
                sB = spool.tile([1, CH], f16, tag=f"sB{b}", name=f"sB{b}")
                nc.vector.tensor_scalar_mul(out=sB[:, :],
                                            in0=acc[0:1, CH:MM_F], scalar1=1.0)
                # transpose row -> columns via K=1 matmuls; the (q even |
                # q odd) fold happens in the PSUM accumulation of the pair
                for k in range(2):
                    mtp = mtpool.tile([P, 1], f32, tag=f"mt{k}", name=f"mt{k}")
                    nc.tensor.matmul(mtp[:, :], lhsT=sA[0:1, k * P:(k + 1) * P],
                                     rhs=ones16[0:1, 0:1], start=True, stop=False)
                    nc.tensor.matmul(mtp[:, :], lhsT=sB[0:1, k * P:(k + 1) * P],
                                     rhs=ones16[0:1, 0:1], start=False, stop=True)
                    part = dv if k == 0 else sc
                    nc.vector.tensor_add(out=m16[k][:, b:b + 1],
                                         in0=mtp[:, :], in1=part[:, :])

            # ---- joint MLP for both batches, [128, BL] wide
            def leaky(hp, bias_col, tag):
                z = spool.tile([P, BL], f16, tag=f"z{tag}", name=f"z{tag}")
                nc.scalar.activation(z[:, :], hp[:, :], Act.Identity,
                                     bias=biases[:, bias_col:bias_col + 1],
                                     scale=1.0)
                h = spool.tile([P, BL], f16, tag=f"h{tag}", name=f"h{tag}")
                nc.vector.scalar_tensor_tensor(out=h[:, :], in0=z[:, :],
                                               scalar=0.01, in1=z[:, :],
                                               op0=Alu.mult, op1=Alu.max)
                return h

            h1p = mmpool.tile([P, BL], f32, tag="mlp", name="h1p")
            for k in range(2):
                nc.tensor.matmul(h1p[:, :],
                                 lhsT=wpk[:, OFF_W1 + k * P:OFF_W1 + (k + 1) * P],
                                 rhs=m16[k][:, :], start=(k == 0), stop=(k == 1))
            z1 = leaky(h1p, 0, "1")
            h2p = mmpool.tile([P, BL], f32, tag="mlp", name="h2p")
            nc.tensor.matmul(h2p[:, :], lhsT=wpk[:, OFF_WO2:OFF_WO2 + P],
                             rhs=z1[:, :], start=True, stop=True)
            z2 = leaky(h2p, 1, "2")
            orp = mmpool.tile([BL, CH], f32, tag="orp")
            nc.tensor.matmul(orp[:, :], lhsT=z2[:, :],
                             rhs=wpk[:, OFF_WO3:OFF_WO3 + CH],
                             start=True, stop=True)
            orow = spool.tile([BL, CH], f32, tag="orow")
            nc.vector.tensor_add(out=orow[:, :], in0=orp[:, :], in1=b3sb[:, :])
            nc.sync.dma_start(out=out_rows[:, :], in_=orow[:, :])

    nc.compile()
    return nc


def _pack_inputs(inputs):
    f8 = ml_dtypes.float8_e3m4
    lidar8 = np.asarray(inputs["lidar"], np.float32).astype(f8)  # [16,4096,256]
    # point-major region: partition p holds points p*Q_PE .. p*Q_PE+Q_PE-1
    pe8 = lidar8[:, :X_PE].reshape(B, P, PE_F)
    # channel-major region: [b, p, (h q)] = lidar[b, tail point q, ch h*128+p]
    ch8 = (lidar8[:, X_PE:].transpose(0, 2, 1).reshape(B, 2, P, X_V)
           .transpose(0, 2, 1, 3).reshape(B, P, CH_F))
    lid8 = np.ascontiguousarray(np.concatenate([pe8, ch8], axis=2))

    wv = np.asarray(inputs["wv"], np.float64)
    wo1 = np.asarray(inputs["wo1"], np.float64)
    W1 = (wv @ wo1) / NPTS                    # [256, 128]; mean folded in

    wpk = np.zeros((P, WPK_F), np.float16)
    wpk[:, OFF_W1:OFF_W1 + P] = W1[0:P, :]
    wpk[:, OFF_W1 + P:OFF_W1 + 2 * P] = W1[P:2 * P, :]
    wpk[:, OFF_WO2:OFF_WO2 + P] = np.asarray(inputs["wo2"], np.float32)
    wpk[:, OFF_WO3:OFF_WO3 + CH] = np.asarray(inputs["wo3"], np.float32)
    bb = np.stack([np.asarray(inputs["b1"], np.float32),
                   np.asarray(inputs["b2"], np.float32)], axis=1)  # [128, 2]
    wpk[:, OFF_B:OFF_B + 4] = bb.view(np.float16)
    b3h = np.asarray(inputs["b3"], np.float32).reshape(1, CH).view(np.float16)
    wpk[0:2, OFF_B3:OFF_B3 + 2 * CH] = np.broadcast_to(b3h, (2, 2 * CH))
    return lid8, wpk


def kernel(**inputs):
    from concourse.bass_utils import run_bass_kernel_spmd

    if "nc" not in _CACHE:
        _CACHE["nc"] = _build_program()
    nc = _CACHE["nc"]

    lid8, wpk = _pack_inputs(inputs)

    in_maps = [
        {"lid8": lid8[i * BL:(i + 1) * BL], "wpk": wpk}
        for i in range(N_CORES)
    ]
    res = run_bass_kernel_spmd(nc, in_maps, list(range(N_CORES)),
                               **_CACHE.get("run_kwargs", {}))
    _CACHE["last_results"] = res
    out = np.concatenate([res.results[i]["out_rows"] for i in range(N_CORES)], axis=0)
    return np.ascontiguousarray(out, dtype=np.float32)


# revision 14
# speedup vs baseline: 1.2742x; 1.0175x over previous
"""Trainium2 Bass kernel for nn_Cross_Attention_Block_3624952397825.

Mathematical structure exploited: the reference takes ``out[:, -1, :]`` --
the attention output of the LAST query token. That token comes from the
zero row appended by ``jnp.pad`` AFTER the conv stack, so its query vector
is exactly zero, its attention scores are exactly zero, and softmax over
exact zeros is exactly uniform (1/4096).  Hence

    bins[b] = mean_k V[b, k, :] = (mean_k lidar[b, k, :]) @ wv
    out[b]  = MLP3(leaky_relu chain)(bins[b])

The conv block, Q/K projections, and softmax are structurally dead code
for ANY input values.  There is no nonlinearity between wv and wo1, so
W1 = (wv @ wo1) / NPTS is constant-folded on the host (the 1/4096 mean
scale riding along).

Implementation (per core, 2 batches, fp8 e3m4 lidar = 2 MiB streamed on
the single SP HWDGE queue):
  * Point-major tiles [128 pts, (q c)] reduced on TensorE via ones^T
    matmuls (fp8e3, fp32 PSUM accumulation).  Warm-up matmuls hold the
    PE busy through the preamble so it is out of its low p-state when
    real data lands.
  * Channel-major tiles [128 ch, pts]: DVE tensor_reduce handles channel
    half 0, ScalarE activation(accum_out=) handles half 1 -- both produce
    [128, 1] column partial sums directly (no transpose needed).
  * Tail is 2-wide across batches: fold + K=1 transpose matmuls + 3-layer
    MLP (ScalarE Identity+bias, DVE leaky), one [2, 256] output DMA.
  * Unused DMA queue sets (Pool SWDGE, Activation HWDGE) are dropped from
    the module so the NEFF prologue/teardown has fewer rings to init and
    drain -- the per-ring drain loop at program end is a large fixed cost.
fp8 quantization of lidar gives rel err ~1.2e-2 (< 2e-2 gate); everything
downstream of the sums is fp16/fp32.
"""

import numpy as np
import ml_dtypes

B, NPTS, CH = 16, 4096, 256
N_CORES = 8
BL = B // N_CORES            # batches per core
P = 128

# Per-batch split of the 4096 points:
#   X_PE point-major -> TensorE;  X_V channel-major -> DVE (ch 0-127) +
#   ScalarE (ch 128-255).  Rates (sustained): PE ~0.89 ns/row,
#   DVE ~1.39 ns/elem (fp8 1x), ACT ~1.27 ns/elem.
Q_PE = 12                    # points per partition in the PE region
X_PE = P * Q_PE              # 1792
X_V = NPTS - X_PE            # 2304
PE_F = Q_PE * CH             # 3584 free elements of PE data per batch
CH_F = 2 * X_V               # 4608 free elements of channel-major data
TILE_F = PE_F + CH_F         # 8192 per batch (1 MiB fp8)
MM_F = 2 * CH                # 512-wide moving operand per matmul
N_WARM = 7                   # PE p-state warm-up matmuls
STRIP_QUEUES = True          # drop Pool/Act DMA queue sets from the module

# fp16 weight pack layout (free dim of wpk)
OFF_W1 = 0                   # 2 k-chunks x 128   (W1 = wv @ wo1 / NPTS)
OFF_WO2 = 256                # 128
OFF_WO3 = 384                # 256
OFF_B = 640                  # 4 fp16 cols = [b1, b2] bitcast fp32 [128, 2]
OFF_B3 = 644                 # rows 0-1, 512 fp16 cols = b3 bitcast [2, 256]
WPK_F = 1156

_CACHE = {}


def _build_program():
    import concourse.bacc as bacc
    import concourse.mybir as mybir
    from concourse.tile import TileContext

    f32 = mybir.dt.float32
    f16 = mybir.dt.float16
    f8 = mybir.dt.float8e3
    Alu = mybir.AluOpType
    Act = mybir.ActivationFunctionType
    Ax = mybir.AxisListType

    nc = bacc.Bacc("TRN2")
    if STRIP_QUEUES:
        nc.m.queues = [q for q in nc.m.queues if q.name == "qSPDynamicHW"]

    lid = nc.dram_tensor("lid8", [BL, P, TILE_F], f8, kind="ExternalInput")
    wpkd = nc.dram_tensor("wpk", [P, WPK_F], f16, kind="ExternalInput")
    out_rows = nc.dram_tensor("out_rows", [BL, CH], f32, kind="ExternalOutput")

    with TileContext(nc) as tc:
        with (
            tc.tile_pool(name="w", bufs=1) as wpool,
            tc.tile_pool(name="io", bufs=1) as iopool,
            tc.tile_pool(name="small", bufs=1) as spool,
            tc.tile_pool(name="acc", bufs=1, space="PSUM") as accpool,
            tc.tile_pool(name="mt", bufs=1, space="PSUM") as mtpool,
            tc.tile_pool(name="mm", bufs=1, space="PSUM") as mmpool,
        ):
            # memsets on GpSimd: it is free right after the preamble, while
            # DVE/ACT are still loading their engine tables
            ones8 = wpool.tile([P, 1], f8, tag="ones8")
            nc.gpsimd.memset(ones8[:, :], 1.0)
            ones16 = wpool.tile([1, 1], f16, tag="ones16")
            nc.gpsimd.memset(ones16[:, :], 1.0)

            # ---- PE p-state warm-up: garbage matmuls on a zeroed tile
            warm = wpool.tile([P, MM_F], f16, tag="warm")
            nc.gpsimd.memset(warm[:, :], 0.0)
            wps = mmpool.tile([1, MM_F], f32, tag="warm")
            for _ in range(N_WARM):
                nc.tensor.matmul(wps[:, :], lhsT=warm[:, 0:1], rhs=warm[:, :],
                                 start=True, stop=True)

            # ---- the full lidar stream on the SP queue; weights ride last
            tiles = []
            for b in range(BL):
                tch = iopool.tile([P, CH_F], f8, tag=f"tch{b}", name=f"tch{b}")
                nc.sync.dma_start(out=tch[:, :], in_=lid[b, :, PE_F:TILE_F])
                tpe = iopool.tile([P, PE_F], f8, tag=f"tpe{b}", name=f"tpe{b}")
                nc.sync.dma_start(out=tpe[:, :], in_=lid[b, :, 0:PE_F])
                tiles.append((tpe, tch))
            wpk = wpool.tile([P, WPK_F], f16, tag="wpk")
            nc.sync.dma_start(out=wpk[:, :], in_=wpkd[:, :])
            biases = wpk[:, OFF_B:OFF_B + 4].bitcast(f32)    # [128, 2] b1,b2
            b3sb = wpk[0:2, OFF_B3:OFF_B3 + 2 * CH].bitcast(f32)  # [2, 256]

            # ---- per-batch reductions
            m16 = [spool.tile([P, BL], f16, tag=f"m16{k}", name=f"m16{k}")
                   for k in range(2)]
            accs, dvs, scs = [], [], []
            for b in range(BL):
                tpe, tch = tiles[b]
                acc = accpool.tile([1, MM_F], f32, tag=f"acc{b}", name=f"acc{b}")
                nmm = PE_F // MM_F
                for j in range(nmm):
                    nc.tensor.matmul(acc[:, :], lhsT=ones8[:, :],
                                     rhs=tpe[:, j * MM_F:(j + 1) * MM_F],
                                     start=(j == 0), stop=(j == nmm - 1))
                # channel-major partial sums: DVE ch 0-127, ACT ch 128-255
                dv = spool.tile([P, 1], f32, tag=f"dv{b}", name=f"dv{b}")
                nc.vector.tensor_reduce(out=dv[:, :], in_=tch[:, 0:X_V],
                                        axis=Ax.X, op=Alu.add)
                sc = spool.tile([P, 1], f32, tag=f"sc{b}", name=f"sc{b}")
                nc.scalar.activation(tch[:, X_V:CH_F], tch[:, X_V:CH_F],
                                     Act.Copy, accum_out=sc[:, :])
                accs.append(acc); dvs.append(dv); scs.append(sc)
            for b in range(BL):
                acc, dv, sc = accs[b], dvs[b], scs[b]
                # PSUM -> SBUF bounce, split across ACT and DVE in parallel
                sA = spool.tile([1, CH], f16, tag=f"sA{b}", name=f"sA{b}")
                nc.scalar.copy(out=sA[:, :], in_=acc[0:1, 0:CH])
                sB = spool.tile([1, CH], f16, tag=f"sB{b}", name=f"sB{b}")
                nc.vector.tensor_scalar_mul(out=sB[:, :],
                                            in0=acc[0:1, CH:MM_F], scalar1=1.0)
                # transpose row -> columns via K=1 matmuls; the (q even |
                # q odd) fold happens in the PSUM accumulation of the pair
                for k in range(2):
                    mtp = mtpool.tile([P, 1], f32, tag=f"mt{k}", name=f"mt{k}")
                    nc.tensor.matmul(mtp[:, :], lhsT=sA[0:1, k * P:(k + 1) * P],
                                     rhs=ones16[0:1, 0:1], start=True, stop=False)
                    nc.tensor.matmul(mtp[:, :], lhsT=sB[0:1, k * P:(k + 1) * P],
                                     rhs=ones16[0:1, 0:1], start=False, stop=True)
                    part = dv if k == 0 else sc
                    nc.vector.tensor_add(out=m16[k][:, b:b + 1],
                                         in0=mtp[:, :], in1=part[:, :])

            # ---- joint MLP for both batches, [128, BL] wide
            def leaky(hp, bias_col, tag):
                z = spool.tile([P, BL], f16, tag=f"z{tag}", name=f"z{tag}")
                nc.scalar.activation(z[:, :], hp[:, :], Act.Identity,
                                     bias=biases[:, bias_col:bias_col + 1],
                                     scale=1.0)
                h = spool.tile([P, BL], f16, tag=f"h{tag}", name=f"h{tag}")
                nc.vector.scalar_tensor_tensor(out=h[:, :], in0=z[:, :],
                                               scalar=0.01, in1=z[:, :],
                                               op0=Alu.mult, op1=Alu.max)
                return h

            h1p = mmpool.tile([P, BL], f32, tag="mlp", name="h1p")
            for k in range(2):
                nc.tensor.matmul(h1p[:, :],
                                 lhsT=wpk[:, OFF_W1 + k * P:OFF_W1 + (k + 1) * P],
                                 rhs=m16[k][:, :], start=(k == 0), stop=(k == 1))
            z1 = leaky(h1p, 0, "1")
            h2p = mmpool.tile([P, BL], f32, tag="mlp", name="h2p")
            nc.tensor.matmul(h2p[:, :], lhsT=wpk[:, OFF_WO2:OFF_WO2 + P],
                             rhs=z1[:, :], start=True, stop=True)
            z2 = leaky(h2p, 1, "2")
            orp = mmpool.tile([BL, CH], f32, tag="orp")
            nc.tensor.matmul(orp[:, :], lhsT=z2[:, :],
                             rhs=wpk[:, OFF_WO3:OFF_WO3 + CH],
                             start=True, stop=True)
            orow = spool.tile([BL, CH], f32, tag="orow")
            nc.vector.tensor_add(out=orow[:, :], in0=orp[:, :], in1=b3sb[:, :])
            nc.sync.dma_start(out=out_rows[:, :], in_=orow[:, :])

    nc.compile()
    return nc


def _pack_inputs(inputs):
    f8 = ml_dtypes.float8_e3m4
    lidar8 = np.asarray(inputs["lidar"], np.float32).astype(f8)  # [16,4096,256]
    # point-major region: partition p holds points p*Q_PE .. p*Q_PE+Q_PE-1
    pe8 = lidar8[:, :X_PE].reshape(B, P, PE_F)
    # channel-major region: [b, p, (h q)] = lidar[b, tail point q, ch h*128+p]
    ch8 = (lidar8[:, X_PE:].transpose(0, 2, 1).reshape(B, 2, P, X_V)
           .transpose(0, 2, 1, 3).reshape(B, P, CH_F))
    lid8 = np.ascontiguousarray(np.concatenate([pe8, ch8], axis=2))

    wv = np.asarray(inputs["wv"], np.float64)
    wo1 = np.asarray(inputs["wo1"], np.float64)
    W1 = (wv @ wo1) / NPTS                    # [256, 128]; mean folded in

    wpk = np.zeros((P, WPK_F), np.float16)
    wpk[:, OFF_W1:OFF_W1 + P] = W1[0:P, :]
    wpk[:, OFF_W1 + P:OFF_W1 + 2 * P] = W1[P:2 * P, :]
    wpk[:, OFF_WO2:OFF_WO2 + P] = np.asarray(inputs["wo2"], np.float32)
    wpk[:, OFF_WO3:OFF_WO3 + CH] = np.asarray(inputs["wo3"], np.float32)
    bb = np.stack([np.asarray(inputs["b1"], np.float32),
                   np.asarray(inputs["b2"], np.float32)], axis=1)  # [128, 2]
    wpk[:, OFF_B:OFF_B + 4] = bb.view(np.float16)
    b3h = np.asarray(inputs["b3"], np.float32).reshape(1, CH).view(np.float16)
    wpk[0:2, OFF_B3:OFF_B3 + 2 * CH] = np.broadcast_to(b3h, (2, 2 * CH))
    return lid8, wpk


def kernel(**inputs):
    from concourse.bass_utils import run_bass_kernel_spmd

    if "nc" not in _CACHE:
        _CACHE["nc"] = _build_program()
    nc = _CACHE["nc"]

    lid8, wpk = _pack_inputs(inputs)

    in_maps = [
        {"lid8": lid8[i * BL:(i + 1) * BL], "wpk": wpk}
        for i in range(N_CORES)
    ]
    res = run_bass_kernel_spmd(nc, in_maps, list(range(N_CORES)),
                               **_CACHE.get("run_kwargs", {}))
    _CACHE["last_results"] = res
    out = np.concatenate([res.results[i]["out_rows"] for i in range(N_CORES)], axis=0)
    return np.ascontiguousarray(out, dtype=np.float32)
